# revision 1
# baseline (speedup 1.0000x reference)
"""Trainium2 Bass kernel for nn_Attention_CA (sparse_attention).

Reference computation (NUM_HEADS=8):
    x_pool = avgpool4(kv)                  # [b, 96, 4096]
    q = l2norm(Q.reshape(b, 8, 48, 65536)) # over last axis
    k = v = l2norm(x_pool.reshape(b, 8, 12, 4096))
    k, v tiled 16x along length -> 65536
    attn = softmax(q @ k^T)                # [b, 8, 48, 12]
    out  = attn @ v                        # [b, 8, 48, 65536]
    y    = W_proj @ out                    # 1x1 conv over channels

Algebraic structure exploited:
  * q @ tile(k,16)^T == fold16(q) @ k^T where fold16 sums the 16 length-4096
    chunks of each q row.  The q l2-normalisation is a per-row scalar, so it
    becomes a per-row scale of the logits (a softmax temperature).
  * attn @ tile(v,16) is 16x periodic along the length dim, and so is the
    1x1 projection of it.  The device therefore only produces y_small
    [2, 384, 4096]; the host materialises the full [2, 384, 256, 256] output
    by tiling (exact, not an approximation).

Sharding over 8 cores: core i owns batch b=i//4 and heads {2a, 2a+1}
(a = i%4), i.e. 96 q-channel rows, 96 raw kv rows (-> 24 pooled rows) and 96
output channels of W_proj.  The only cross-core exchange is an AllGather of
the per-core attention outputs [96, 4096] within each batch group of 4
cores, ahead of the channel-contracting 1x1 projection.
"""

import numpy as np

NUM_HEADS = 8
B, C, H, W = 2, 384, 256, 256
HW = H * W           # 65536
L = 4096             # kv length == pooled row length
J = HW // L          # 16 fold chunks
CQ = C // NUM_HEADS  # 48 q rows per head
ROWS = 96            # q rows per core (2 heads)
KR = 24              # pooled kv rows per core (2 heads x 12)
NCORES = 8
GROUP = 4            # cores per batch
EPS = 1e-12

_CACHE = {}


def _build():
    import os as _os
    import concourse.bacc as bacc
    import concourse.mybir as mybir
    from concourse.tile import TileContext

    STAGE = int(_os.environ.get("KERNEL_STAGE", "4"))
    f32 = mybir.dt.float32
    bf16 = mybir.dt.bfloat16
    Alu = mybir.AluOpType
    Act = mybir.ActivationFunctionType

    nc = bacc.Bacc(num_devices=NCORES)

    MQ = L // NCORES     # 512: m-slice each core projects
    q_in = nc.dram_tensor("q", [ROWS, J, L], f32, kind="ExternalInput")
    kv_in = nc.dram_tensor("kv", [ROWS, L], f32, kind="ExternalInput")
    w_in = nc.dram_tensor("w", [C, C], f32, kind="ExternalInput")
    y_out = nc.dram_tensor("y", [B, C, MQ], f32, kind="ExternalOutput")

    # constants baked into the NEFF
    ident_np = np.eye(128, dtype=np.float32)
    poolmat_np = np.zeros((ROWS, KR), dtype=np.float32)
    for k in range(KR):
        poolmat_np[4 * k:4 * k + 4, k] = 0.25
    # block-diagonal head mask: head0 rows see cols 0:12, head1 rows 12:24
    mask_np = np.zeros((ROWS, KR), dtype=np.float32)
    mask_np[:CQ, :12] = 1.0
    mask_np[CQ:, 12:] = 1.0
    ident_dram = nc.inline_tensor(ident_np, name="ident")
    poolmat_dram = nc.inline_tensor(poolmat_np, name="poolmat")
    mask_dram = nc.inline_tensor(mask_np, name="mask")

    # collective bounce buffers (8-core AllToAll: m-eighths <-> channels)
    so_dram = nc.dram_tensor("so_local", [NCORES * ROWS * MQ], bf16)
    a2a_dram = nc.dram_tensor("so_a2a", [NCORES * ROWS * MQ], bf16)

    NT = L // 128        # 32 column tiles of the folded q
    NMM = L // 512       # 8 matmul column chunks

    with TileContext(nc) as tc:
        with (
            tc.tile_pool(name="big", bufs=3) as big_pool,
            tc.tile_pool(name="persist", bufs=1) as persist,
            tc.tile_pool(name="small", bufs=2) as small,
        ):
            with tc.tile_pool(name="psum", bufs=2, space="PSUM") as psum:
                ident = persist.tile([128, 128], f32)
                nc.sync.dma_start(out=ident, in_=ident_dram[:, :])
                poolmat = persist.tile([ROWS, KR], f32)
                nc.sync.dma_start(out=poolmat, in_=poolmat_dram[:, :])

                # ---- kv: pool -> l2-normalise (independent of Q) ----
                kv_sb = big_pool.tile([ROWS, L], f32, tag="kv", bufs=1)
                nc.sync.dma_start(out=kv_sb, in_=kv_in[:, :])
                kn = persist.tile([KR, L], f32)
                for n in range(NMM):
                    ppool = psum.tile([KR, 512], f32, tag="mm512")
                    nc.tensor.matmul(ppool, lhsT=poolmat,
                                     rhs=kv_sb[:, n * 512:(n + 1) * 512],
                                     start=True, stop=True)
                    nc.scalar.copy(kn[:, n * 512:(n + 1) * 512], ppool)
                ksq = small.tile([KR, 1], f32)
                ksc = big_pool.tile([KR, L], f32, tag="sqscr", bufs=1)
                nc.scalar.activation(ksc, kn, Act.Square, accum_out=ksq)
                knrm = small.tile([KR, 1], f32)
                nc.scalar.sqrt(knrm, ksq)
                nc.vector.tensor_scalar_max(knrm, knrm, EPS)
                kinv = small.tile([KR, 1], f32)
                nc.vector.reciprocal(kinv, knrm)
                nc.vector.tensor_scalar_mul(kn, kn, kinv)
                # bf16 copy of normalized kn for the small_out matmul
                kn_bf = persist.tile([KR, L], bf16)
                nc.vector.tensor_copy(kn_bf, kn)

                # ---- full W, transposed: K-chunks of 96, O-blocks of 128 --
                NB = C // 128   # 3 output-row blocks
                NK = GROUP      # 4 channel chunks of 96 (align a2a blocks)
                w_sb = persist.tile([128, NB, C], f32)
                nc.sync.dma_start(
                    out=w_sb,
                    in_=w_in[:, :].rearrange("(b p) c -> p b c", p=128))
                wT = persist.tile([ROWS, NK, NB, 128], bf16)
                for kc in range(NK):
                    for ob in range(NB):
                        pw = psum.tile([ROWS, 128], f32, tag="tp")
                        nc.tensor.transpose(
                            pw, w_sb[:, ob, kc * ROWS:(kc + 1) * ROWS],
                            ident)
                        nc.scalar.copy(wT[:, kc, ob, :], pw)

                # transpose kn -> knT [128, NT, KR]
                knT = persist.tile([128, NT, KR], f32)
                for t in range(NT):
                    pt = psum.tile([128, KR], f32, tag="tp")
                    nc.tensor.transpose(pt, kn[:, t * 128:(t + 1) * 128],
                                        ident[:KR, :KR])
                    nc.scalar.copy(knT[:, t, :], pt)

                # ---- Q: fold 16 chunks + sum of squares ----
                acc = persist.tile([ROWS, L], f32)
                sqparts = persist.tile([ROWS, J], f32)
                for j in range(J):
                    chunk = big_pool.tile([ROWS, L], f32, tag="chunk",
                                          bufs=4, name=f"chunk{j}")
                    nc.sync.dma_start(out=chunk[:, :L // 2],
                                      in_=q_in[:, j, :L // 2])
                    nc.sync.dma_start(out=chunk[:, L // 2:],
                                      in_=q_in[:, j, L // 2:])
                    if j == 0:
                        nc.vector.tensor_copy(acc, chunk)
                    else:
                        nc.vector.tensor_add(acc, acc, chunk)
                    sqscr = big_pool.tile([ROWS, L], f32, tag="sqscr", bufs=1)
                    nc.scalar.activation(sqscr, chunk, Act.Square,
                                         accum_out=sqparts[:, j:j + 1])

                sumsq = small.tile([ROWS, 1], f32)
                nc.vector.reduce_sum(sumsq, sqparts, axis=mybir.AxisListType.X)
                qnrm = small.tile([ROWS, 1], f32)
                nc.scalar.sqrt(qnrm, sumsq)
                nc.vector.tensor_scalar_max(qnrm, qnrm, EPS)
                qinv = small.tile([ROWS, 1], f32)
                nc.vector.reciprocal(qinv, qnrm)

                # transpose acc -> qfT [128, NT, ROWS]
                qfT = persist.tile([128, NT, ROWS], f32)
                for t in range(NT):
                    ptq = psum.tile([128, ROWS], f32, tag="tp")
                    nc.tensor.transpose(ptq, acc[:, t * 128:(t + 1) * 128],
                                        ident[:ROWS, :ROWS])
                    nc.scalar.copy(qfT[:, t, :], ptq)

                # ---- attention logits for both heads in one matmul chain ---
                # out[96, 24]: block diag [48x12 | 48x12] is valid, rest junk
                pattn = psum.tile([ROWS, KR], f32, tag="attn", bufs=1)
                for t in range(NT):
                    nc.tensor.matmul(pattn, lhsT=qfT[:, t, :], rhs=knT[:, t, :],
                                     start=(t == 0), stop=(t == NT - 1))

                # ---- softmax (no max-subtraction: |logits| <= 4) ----
                # Scale+exp the full [96, 24] (off-block junk is bounded),
                # then mask block-diagonally while row-summing in one DVE op.
                mask_sb = persist.tile([ROWS, KR], f32)
                nc.sync.dma_start(out=mask_sb, in_=mask_dram[:, :])
                e_sb = small.tile([ROWS, KR], f32)
                nc.vector.tensor_scalar(e_sb, pattn, qinv, None, Alu.mult)
                nc.scalar.activation(e_sb, e_sb, Act.Exp)
                p_sb = small.tile([ROWS, KR], f32)
                nc.vector.tensor_mul(p_sb, e_sb, mask_sb)
                esum = small.tile([ROWS, 1], f32)
                nc.vector.reduce_sum(esum, p_sb, axis=mybir.AxisListType.X)
                einv = small.tile([ROWS, 1], f32)
                nc.vector.reciprocal(einv, esum)

                # one PE transpose yields block-diagonal pT [24, 96]
                pT = small.tile([KR, ROWS], bf16)
                ptp = psum.tile([KR, ROWS], f32, tag="tp")
                nc.tensor.transpose(ptp, p_sb, ident[:ROWS, :ROWS])
                nc.scalar.copy(pT, ptp)

                # ---- small_out = softmax(p) @ kn (both heads at once) ----
                # the 1/sum(exp) row scale is applied on the PSUM->SBUF copy
                so_sb = persist.tile([ROWS, L], bf16)
                for n in range(NMM):
                    pso = psum.tile([ROWS, 512], f32, tag="mm512")
                    nc.tensor.matmul(pso, lhsT=pT,
                                     rhs=kn_bf[:, n * 512:(n + 1) * 512],
                                     start=True, stop=True)
                    nc.scalar.activation(so_sb[:, n * 512:(n + 1) * 512], pso,
                                         Act.Copy, scale=einv)

                # ---- 8-core AllToAll: shard r = so[:, 512r:512r+512] ----
                # received block g = [96 channel rows of batch g//4,
                # channel block g%4, my m-eighth]
                nc.sync.dma_start(
                    out=so_dram[:].rearrange("(g p m) -> p g m",
                                             g=NCORES, p=ROWS),
                    in_=so_sb.rearrange("p (g m) -> p g m", g=NCORES))
                nc.gpsimd.collective_compute(
                    "AllToAll", Alu.bypass,
                    replica_groups=[[0, 1, 2, 3, 4, 5, 6, 7]],
                    ins=[so_dram[:]],
                    outs=[a2a_dram[:]],
                )

            # ---- projection: y[b, :, my m-eighth] = W @ so_all[b] ----
            # first PSUM pool released; 6 banks as accumulators
            with tc.tile_pool(name="psum_y", bufs=1, space="PSUM") as psum_y:
                a2a_ap = a2a_dram[:].rearrange("(g p m) -> g p m",
                                               g=NCORES, p=ROWS)
                py = [[psum_y.tile([128, MQ], f32, tag=f"y{b}{ob}",
                                   name=f"py{b}{ob}") for ob in range(NB)]
                      for b in range(B)]
                for b in range(B):
                    for kc in range(NK):
                        gt = big_pool.tile([ROWS, MQ], bf16, tag="gath",
                                           bufs=2)
                        nc.sync.dma_start(out=gt,
                                          in_=a2a_ap[GROUP * b + kc, :, :])
                        for ob in range(NB):
                            nc.tensor.matmul(
                                py[b][ob], lhsT=wT[:, kc, ob, :], rhs=gt,
                                start=(kc == 0), stop=(kc == NK - 1))
                for b in range(B):
                    y_ap = y_out[b, :, :].rearrange("(ob p) m -> p ob m",
                                                    p=128)
                    for ob in range(NB):
                        y_sb = small.tile([128, MQ], f32, tag="ysb")
                        nc.scalar.copy(y_sb, py[b][ob])
                        nc.sync.dma_start(out=y_ap[:, ob, :], in_=y_sb)

    if not nc.is_finalized():
        nc.finalize()
    return nc


def _get_nc():
    if "nc" not in _CACHE:
        _CACHE["nc"] = _build()
    return _CACHE["nc"]


def kernel(Q, kv, W_proj, _trace=False):
    from concourse.bass_utils import run_bass_kernel_spmd

    Q = np.ascontiguousarray(Q, dtype=np.float32)
    kv = np.ascontiguousarray(kv, dtype=np.float32)
    W_proj = np.ascontiguousarray(W_proj, dtype=np.float32)

    Qr = Q.reshape(B * C, J, L)
    in_maps = []
    for i in range(NCORES):
        b, a = divmod(i, GROUP)
        sl = slice(96 * a, 96 * a + 96)
        in_maps.append({
            "q": np.ascontiguousarray(Qr[b * C + 96 * a: b * C + 96 * a + 96]),
            "kv": np.ascontiguousarray(kv[b, sl]),
            "w": W_proj,
        })

    nc = _get_nc()
    res = run_bass_kernel_spmd(nc, in_maps, core_ids=list(range(NCORES)),
                               trace=_trace)
    _CACHE["last_results"] = res

    MQ = L // NCORES
    y_small = np.empty((B, C, L), np.float32)
    for i in range(NCORES):
        y_small[:, :, MQ * i: MQ * (i + 1)] = res.results[i]["y"]

    out = np.broadcast_to(y_small[:, :, None, :], (B, C, J, L))
    return np.ascontiguousarray(out).reshape(B, C, H, W)



# revision 31
# speedup vs baseline: 1.4719x; 1.4719x over previous
"""Trainium2 Bass kernel for nn_Attention_CA (sparse_attention).

Reference computation (NUM_HEADS=8):
    x_pool = avgpool4(kv)                  # [b, 96, 4096]
    q = l2norm(Q.reshape(b, 8, 48, 65536)) # over last axis
    k = v = l2norm(x_pool.reshape(b, 8, 12, 4096))
    k, v tiled 16x along length -> 65536
    attn = softmax(q @ k^T)                # [b, 8, 48, 12]
    out  = attn @ v                        # [b, 8, 48, 65536]
    y    = W_proj @ out                    # 1x1 conv over channels

Algebraic structure exploited:
  * q @ tile(k,16)^T == fold16(q) @ k^T where fold16 sums the 16 length-4096
    chunks of each q row; the q l2-norm is a per-row softmax temperature.
  * attn @ tile(v,16) is 16x periodic, and so is its 1x1 projection: the
    device produces y_small [2, 384, 4096]; the host tiles it (exact).
  * y = W @ (P @ kn) = (W @ P) @ kn.  P is block-diagonal per head, so the
    rows of WP^T for this core's heads depend only on LOCAL data:
    WPT[k, o] = sum_c P[c, k] W[o, c] over this core's 96 channels.  The
    only tail collective is an 18KB AllGather of WPT, after which each core
    projects its own 512-column slice:  y[b, :, m] = WPT_b^T @ kn_all[b].

Sharding over 8 cores: core i owns batch b=i//4 and heads {2a, 2a+1}
(a = i%4): 96 q-channel rows, 96 raw kv rows (-> 24 pooled rows), and it
produces y[:, :, 512i:512(i+1)].  Collectives: an AllToAll of normalized kn
column-slices (early, fully overlapped with the Q load) and the 18KB WPT
AllGather in the tail.

All device inputs are bf16 (host casts; internal mixed precision, f32
accumulation in PSUM / for sums of squares).  Measured end-to-end rel err
~5e-3 vs the f32 reference (gate: 2e-2).
"""

import numpy as np

NUM_HEADS = 8
B, C, H, W = 2, 384, 256, 256
HW = H * W           # 65536
L = 4096             # kv length == pooled row length
J = 16               # fold chunks
ROWS = 96            # q rows per core (2 heads)
KR = 24              # pooled kv rows per core
NCORES = 8
MQ = L // NCORES     # 512 output columns per core
NBLK = 4             # column blocks of the fold pipeline
MBLK = L // NBLK     # 1024 columns per block
EPS = 1e-12

# engine split for the per-chunk elementwise work (tuned from traces)
GPSIMD_FOLD_J = frozenset({3, 6, 9, 12, 15})   # fold adds done on gpsimd
DVE_SQ_J = frozenset({0, 2, 4, 6, 8, 10, 12, 14})  # squares on DVE, rest Act

_CACHE = {}


def _build():
    import concourse.bacc as bacc
    import concourse.mybir as mybir
    from concourse.tile import TileContext

    f32 = mybir.dt.float32
    bf16 = mybir.dt.bfloat16
    Alu = mybir.AluOpType
    Act = mybir.ActivationFunctionType

    nc = bacc.Bacc(num_devices=NCORES)

    q_in = nc.dram_tensor("q", [NBLK, J // 4, ROWS, 4 * MBLK], bf16, kind="ExternalInput")
    kv_in = nc.dram_tensor("kv", [ROWS, L], bf16, kind="ExternalInput")
    wt_in = nc.dram_tensor("wt", [ROWS, C], bf16, kind="ExternalInput")
    y_out = nc.dram_tensor("y", [B, C, MQ], bf16, kind="ExternalOutput")

    # constants baked into the NEFF
    identq_np = np.eye(ROWS, dtype=np.float32)
    identk_np = np.eye(KR, dtype=np.float32)
    poolmat_np = np.zeros((ROWS, KR), dtype=np.float32)
    for k in range(KR):
        poolmat_np[4 * k:4 * k + 4, k] = 0.25
    # block-diagonal head mask: head0 rows see cols 0:12, head1 rows 12:24
    mask_np = np.zeros((ROWS, KR), dtype=np.float32)
    mask_np[:48, :12] = 1.0
    mask_np[48:, 12:] = 1.0
    import ml_dtypes
    nbf = ml_dtypes.bfloat16
    identq_dram = nc.inline_tensor(identq_np.astype(nbf), name="identq")
    identk_dram = nc.inline_tensor(identk_np.astype(nbf), name="identk")
    poolmat_dram = nc.inline_tensor(poolmat_np.astype(nbf), name="poolmat")
    mask_dram = nc.inline_tensor(mask_np, name="mask")

    # collective bounce buffers
    kn_cc_in = nc.dram_tensor("kn_cc_in", [NCORES * KR * MQ], bf16)
    kn_cc_out = nc.dram_tensor("kn_cc_out", [NCORES * KR * MQ], bf16)
    wpt_cc_in = nc.dram_tensor("wpt_cc_in", [KR * C], bf16)
    wpt_cc_out = nc.dram_tensor("wpt_cc_out", [NCORES * KR * C], bf16)

    NT = MBLK // 128     # 8 transpose tiles per block

    with TileContext(nc) as tc:
        with (
            tc.tile_pool(name="persist", bufs=1) as persist,
            tc.tile_pool(name="chunks", bufs=6) as chunks,
            tc.tile_pool(name="small", bufs=2) as small,
            tc.tile_pool(name="psum", bufs=2, space="PSUM") as psum,
            tc.tile_pool(name="psum_hold", bufs=1, space="PSUM") as psum_hold,
        ):
            pre0 = []
            for g in range(4):
                ch4 = chunks.tile([ROWS, 4 * MBLK], bf16, tag="chunk",
                                  bufs=10, name=f"pre{g}")
                nc.sync.dma_start(out=ch4, in_=q_in[0, g])
                pre0.append(ch4)
            identq = persist.tile([ROWS, ROWS], bf16)
            nc.sync.dma_start(out=identq, in_=identq_dram[:, :])
            identk = persist.tile([KR, KR], bf16)
            nc.sync.dma_start(out=identk, in_=identk_dram[:, :])
            poolmat = persist.tile([ROWS, KR], bf16)
            nc.sync.dma_start(out=poolmat, in_=poolmat_dram[:, :])
            mask_sb = persist.tile([ROWS, KR], f32)
            nc.sync.dma_start(out=mask_sb, in_=mask_dram[:, :])
            wt_sb = persist.tile([ROWS, C], bf16)
            nc.sync.dma_start(out=wt_sb, in_=wt_in[:, :])

            # ---- kv: pool -> l2-normalise -> exchange column slices ----
            kv_sb = persist.tile([ROWS, L], bf16)
            nc.sync.dma_start(out=kv_sb, in_=kv_in[:, :])
            kn = persist.tile([KR, L], f32)
            for n in range(8):
                ppool = psum.tile([KR, 512], f32, tag="mm512")
                nc.tensor.matmul(ppool, lhsT=poolmat,
                                 rhs=kv_sb[:, n * 512:(n + 1) * 512],
                                 start=True, stop=True)
                nc.scalar.copy(kn[:, n * 512:(n + 1) * 512], ppool)
            ksq = small.tile([KR, 1], f32)
            kscr = persist.tile([KR, L], bf16)
            nc.scalar.activation(kscr, kn, Act.Square, accum_out=ksq)
            knrm = small.tile([KR, 1], f32)
            nc.scalar.sqrt(knrm, ksq)
            nc.vector.tensor_scalar_max(knrm, knrm, EPS)
            kinv = small.tile([KR, 1], f32)
            nc.vector.reciprocal(kinv, knrm)
            kn_bf32 = persist.tile([32, L], bf16)
            nc.gpsimd.memset(kn_bf32, 0.0)
            kn_bf = kn_bf32[0:KR, :]
            nc.scalar.activation(kn_bf, kn, Act.Copy, scale=kinv)

            # AllToAll: peer g gets my kn[:, 512g:512(g+1)]; I receive
            # from peer g its slice for my columns.
            kn_all = persist.tile([96, B, MQ], bf16)

            # knT [128, 32, 32] via one XBAR-transposing DMA (cols 24:32 junk)
            knTx = persist.tile([128, L // 128, 32], bf16)
            nc.scalar.dma_start(out=knTx, in_=kn_bf32, transpose=True)

            # ---- Q loop: fold + sumsq + transpose + logits, pipelined ----
            # fold lanes per block: j%4==0,1 -> DVE, j%4==2 -> PE (identity
            # matmul accumulating in PSUM), j%4==3 -> gpsimd (3 chunks) + DVE
            qfT = persist.tile([128, NBLK * NT * ROWS], bf16)
            sqp = persist.tile([ROWS, NBLK * (J // 4)], f32)
            sq_scr_a = persist.tile([ROWS, 4 * MBLK], bf16)
            sq_scr_v = persist.tile([ROWS, 4 * MBLK], bf16)
            lp = psum_hold.tile([ROWS, NBLK * KR], f32, tag="logits")

            acc_bfs = []

            def xbar_and_logits(blk):
                # XBAR transpose [96,1024] -> [128, 8, 96], then this
                # block's logits matmuls accumulate into lp[:, blk*24:+24].
                # blk 0-2 issue from the Act queue so the in-order sync DMA
                # queue never waits on the fold chain; blk3 uses sync.
                qeng = nc.sync if blk == NBLK - 1 else nc.scalar
                qeng.dma_start(
                    out=qfT[:, blk * NT * ROWS:(blk + 1) * NT * ROWS]
                        .rearrange("p (t r) -> p t r", t=NT),
                    in_=acc_bfs[blk], transpose=True)
                for t in range(NT):
                    nc.tensor.matmul(
                        lp[:, blk * KR:(blk + 1) * KR],
                        lhsT=qfT[:, (blk * NT + t) * ROWS:
                                 (blk * NT + t + 1) * ROWS],
                        rhs=knTx[:, blk * NT + t, 0:KR],
                        start=(t == 0), stop=(t == NT - 1),
                        skip_group_check=True)

            for blk in range(NBLK):
                ch4s = []
                accA = chunks.tile([ROWS, MBLK], bf16, tag="accA", bufs=2)
                accB = chunks.tile([ROWS, MBLK], bf16, tag="accB", bufs=2)
                accD = chunks.tile([ROWS, MBLK], bf16, tag="accD", bufs=2)
                pacc0 = psum.tile([ROWS, 512], f32, tag="pe0")
                pacc1 = psum.tile([ROWS, 512], f32, tag="pe1")
                for g in range(4):
                    if blk == 0:
                        ch4 = pre0[g]
                    else:
                        ch4 = chunks.tile([ROWS, 4 * MBLK], bf16,
                                          tag="chunk", bufs=10)
                        nc.sync.dma_start(out=ch4, in_=q_in[blk, g])
                    ch4s.append(ch4)
                    ln = lambda t, l: t[:, l * MBLK:(l + 1) * MBLK]
                    if g == 1:
                        # pair-start: first op is add(g0, g1), no copies
                        nc.vector.tensor_add(accA, ln(ch4s[0], 0), ln(ch4, 0))
                        nc.vector.tensor_add(accB, ln(ch4s[0], 1), ln(ch4, 1))
                        nc.gpsimd.tensor_add(accD, ln(ch4s[0], 3), ln(ch4, 3))
                    elif g > 1:
                        nc.vector.tensor_add(accA, accA, ln(ch4, 0))
                        nc.vector.tensor_add(accB, accB, ln(ch4, 1))
                        if g == 3:
                            nc.vector.tensor_add(accD, accD, ln(ch4, 3))
                        else:
                            nc.gpsimd.tensor_add(accD, accD, ln(ch4, 3))
                    for h in range(2):
                        nc.tensor.matmul(
                            pacc0 if h == 0 else pacc1,
                            lhsT=identq,
                            rhs=ch4[:, (2 * MBLK) + h * 512:
                                    (2 * MBLK) + (h + 1) * 512],
                            start=(g == 0), stop=(g == 3),
                            skip_group_check=True)
                    # squares (grouped over the 4-chunk tile)
                    gi = blk * 4 + g
                    if gi not in (3, 7, 11, 15):
                        nc.scalar.activation(sq_scr_a, ch4, Act.Square,
                                             accum_out=sqp[:, gi:gi + 1])
                    else:
                        nc.vector.scalar_tensor_tensor(
                            out=sq_scr_v, in0=ch4, scalar=1.0, in1=ch4,
                            op0=Alu.mult, op1=Alu.mult,
                            accum_out=sqp[:, gi:gi + 1])
                # reduce lanes -> acc_bf; PE lane read from PSUM
                t0 = chunks.tile([ROWS, MBLK], bf16, tag="red0", bufs=2)
                nc.vector.tensor_add(t0, accA, accB)
                t1 = chunks.tile([ROWS, MBLK], bf16, tag="red1", bufs=2)
                nc.vector.tensor_add(t1[:, 0:512], accD[:, 0:512], pacc0)
                nc.vector.tensor_add(t1[:, 512:1024], accD[:, 512:1024],
                                     pacc1)
                acc_bf = chunks.tile([ROWS, MBLK], bf16, tag="accf", bufs=3)
                nc.vector.tensor_add(acc_bf, t0, t1)
                acc_bfs.append(acc_bf)
                xbar_and_logits(blk)
                if blk == 1:
                    # feed the collective input once kn_bf is ready; placed
                    # here so it does not block the blk0/1 Q loads on the
                    # in-order sync DMA queue
                    nc.sync.dma_start(
                        out=kn_cc_in[:].rearrange("(g p m) -> p g m",
                                                  g=NCORES, p=KR),
                        in_=kn_bf.rearrange("p (g m) -> p g m", g=NCORES))
                if blk == 2:
                    nc.gpsimd.collective_compute(
                        "AllToAll", Alu.bypass,
                        replica_groups=[list(range(NCORES))],
                        ins=[kn_cc_in[:]],
                        outs=[kn_cc_out[:]],
                    )

            # ---- tail: softmax temperature, softmax, WPT, AllGather, y ----
            nc.sync.dma_start(
                out=kn_all,
                in_=kn_cc_out[:].rearrange("(b p m) -> p b m", b=B, p=96))
            ssum = small.tile([ROWS, 1], f32)
            nc.vector.reduce_sum(ssum, sqp, axis=mybir.AxisListType.X)
            qnrm = small.tile([ROWS, 1], f32)
            nc.scalar.sqrt(qnrm, ssum)
            nc.vector.tensor_scalar_max(qnrm, qnrm, EPS)
            qinv = small.tile([ROWS, 1], f32)
            nc.vector.reciprocal(qinv, qnrm)

            lg = small.tile([ROWS, KR], f32)
            nc.vector.tensor_copy(lg, lp[:, 0:KR])
            for blk in range(1, NBLK):
                nc.vector.tensor_add(lg, lg,
                                     lp[:, blk * KR:(blk + 1) * KR])

            # no max-subtraction: |scaled logits| <= 4
            e_sb = small.tile([ROWS, KR], f32)
            nc.scalar.activation(e_sb, lg, Act.Exp, scale=qinv)
            p_sb = small.tile([ROWS, KR], f32)
            esum = small.tile([ROWS, 1], f32)
            nc.vector.scalar_tensor_tensor(
                out=p_sb, in0=e_sb, scalar=1.0, in1=mask_sb,
                op0=Alu.mult, op1=Alu.mult, accum_out=esum)
            einv = small.tile([ROWS, 1], f32)
            nc.vector.reciprocal(einv, esum)
            p_bf = small.tile([ROWS, KR], bf16)
            nc.scalar.activation(p_bf, p_sb, Act.Copy, scale=einv)

            # WPT[k, o] = sum_c P[c, k] W[o, c]  (exact rows for my heads)
            pw = psum.tile([KR, C], f32, tag="tp", bufs=1)
            nc.tensor.matmul(pw, lhsT=p_bf, rhs=wt_sb, start=True, stop=True)
            wpt_sb = small.tile([KR, C], bf16)
            nc.scalar.copy(wpt_sb, pw)
            nc.sync.dma_start(
                out=wpt_cc_in[:].rearrange("(p m) -> p m", p=KR),
                in_=wpt_sb)
            nc.gpsimd.collective_compute(
                "AllGather", Alu.bypass,
                replica_groups=[list(range(NCORES))],
                ins=[wpt_cc_in[:]],
                outs=[wpt_cc_out[:]],
            )
            wpt_all = persist.tile([96, B, C], bf16)
            nc.sync.dma_start(
                out=wpt_all,
                in_=wpt_cc_out[:].rearrange("(b p m) -> p b m", b=B, p=96))

            # y[b, :, my 512 cols] = WPT_b^T @ kn_all[b]
            for b in range(B):
                for ob in range(3):
                    py = psum.tile([128, MQ], f32, tag="mm512")
                    nc.tensor.matmul(
                        py,
                        lhsT=wpt_all[:, b, ob * 128:(ob + 1) * 128],
                        rhs=kn_all[:, b, :],
                        start=True, stop=True)
                    y_sb = small.tile([128, MQ], bf16, tag="ysb")
                    if (b * 3 + ob) % 2 == 0:
                        nc.scalar.copy(y_sb, py)
                    else:
                        nc.vector.tensor_copy(y_sb, py)
                    y_ap = y_out[b, :, :].rearrange("(ob p) m -> p ob m",
                                                    p=128)
                    nc.sync.dma_start(out=y_ap[:, ob, :], in_=y_sb)

    if not nc.is_finalized():
        nc.finalize()
    return nc


def _get_nc():
    if "nc" not in _CACHE:
        _CACHE["nc"] = _build()
    return _CACHE["nc"]


def _to_bf16(a):
    import ml_dtypes
    return np.asarray(a).astype(ml_dtypes.bfloat16)


def kernel(Q, kv, W_proj, _trace=False):
    from concourse.bass_utils import run_bass_kernel_spmd

    Qb = _to_bf16(Q)          # [2, 384, 256, 256]
    kvb = _to_bf16(kv)        # [2, 384, 4096]
    Wb = _to_bf16(W_proj)     # [384, 384]

    # per core: [4 blk, 4 grp, 96 rows, 4 lanes * 1024 cols]
    Qr = Qb.reshape(B, C, 4, 4, NBLK, MBLK)   # [b, c, grp, lane, blk, m]
    in_maps = []
    for i in range(NCORES):
        b, a = divmod(i, 4)
        rows = slice(96 * a, 96 * a + 96)
        qc = Qr[b, rows].transpose(3, 1, 0, 2, 4)  # [blk, grp, r, lane, m]
        qc = np.ascontiguousarray(qc.reshape(NBLK, 4, 96, 4 * MBLK))
        in_maps.append({
            "q": qc,
            "kv": np.ascontiguousarray(kvb[b, rows]),
            "wt": np.ascontiguousarray(Wb[:, rows].T),
        })

    nc = _get_nc()
    res = run_bass_kernel_spmd(nc, in_maps, core_ids=list(range(NCORES)),
                               trace=_trace)
    _CACHE["last_results"] = res

    y_small = np.empty((B, C, L), np.float32)
    for i in range(NCORES):
        y_small[:, :, MQ * i: MQ * (i + 1)] = \
            res.results[i]["y"].astype(np.float32)

    out = np.broadcast_to(y_small[:, :, None, :], (B, C, J, L))
    return np.ascontiguousarray(out).reshape(B, C, H, W)
